# revision 9
# baseline (speedup 1.0000x reference)
"""Trainium2 Bass kernel for nn_Decoder (7+1 conv-bn-relu stack + global mean).

Self-contained: hardcodes shapes from the problem spec.
kernel(**inputs) takes FULL inputs, shards batch across 8 cores, returns [32, 30].

Design (per core, 4 images, all activations SBUF-resident):
- Activation layout: one big in-place SBUF buffer B [128 part, 131 slots, 258].
  Partition p<64 = channel p of the FIRST row of a row-pair, p>=64 = channel
  p-64 of the SECOND row.  A-layout slot j = rows (2j-1, 2j) (odd first);
  B-layout slot j = rows (2j, 2j+1) (even first).  Layers alternate layouts,
  writing in-place with a trailing physical offset.
- Conv as matmul: out-pair (y, y+1) accumulates 6 f32r matmuls
  [K=128, M=128, N=512] in PSUM (2 out-pairs per PSUM bank), start/stop flags.
- BN+ReLU fused into one ScalarE activation per group: relu(psum*s + t) with
  per-partition scale/bias, written straight into the buffer (next layer's
  input, rounded to f32r).
- Final layer (C->30) uses activation accum_out to produce per-channel row
  sums; a DVE reduce gives per-image channel sums; host divides by H*W.
"""
import sys

sys.path.insert(0, "/opt/trn_rl_repo")

import numpy as np
import concourse.bass as bass
import concourse.tile as tile
from concourse import mybir, bacc

dt = mybir.dt

# problem constants
B, CIN, H, W = 32, 3, 256, 256
C, L, MID = 64, 30, 6
NCORES = 8
BPC = B // NCORES  # images per core
BN_EPS = 1e-5

NSLOT = 131          # physical pair-slots in main buffer
WPAD = 258           # padded row width
NPAIR = H // 2       # 128

# layer schedule: (kind, in_off, out_off); L0 special
# L1..L6 mid convs, L7 final
LAYERS = [
    ("stag", 3, 2),   # L1
    ("clean", 2, 2),  # L2
    ("stag", 2, 1),   # L3
    ("clean", 1, 1),  # L4
    ("stag", 1, 0),   # L5
    ("clean", 0, 0),  # L6
    ("final", 0, None),  # L7
]


# ---------------------------------------------------------------- host packing

def _fold_bn(bias, gamma, beta, mean, var):
    s = gamma / np.sqrt(var + BN_EPS)
    t = (bias - mean) * s + beta
    return s.astype(np.float32), t.astype(np.float32)


def _pair_lhst(w, cout):
    """w [cout, cin, 3, 3] -> (A, B) each [3 dx][128, 128] block lhsT."""
    cin = w.shape[1]
    wd = np.transpose(w, (1, 0, 2, 3)).astype(np.float32)  # [cin, cout, ky, kx]
    A = np.zeros((3, 128, 128), np.float32)
    Bm = np.zeros((3, 128, 128), np.float32)
    for dx in range(3):
        A[dx, 0:cin, 0:cout] = wd[:, :, 0, dx]
        A[dx, 64:64 + cin, 0:cout] = wd[:, :, 1, dx]
        A[dx, 64:64 + cin, 64:64 + cout] = wd[:, :, 0, dx]
        Bm[dx, 0:cin, 0:cout] = wd[:, :, 2, dx]
        Bm[dx, 0:cin, 64:64 + cout] = wd[:, :, 1, dx]
        Bm[dx, 64:64 + cin, 64:64 + cout] = wd[:, :, 2, dx]
    return A, Bm


def _single_lhst(w, cout):
    """Singles for staggered layers: (s0, s255) each [3 dx][128, 128].

    M padded to 128 so the PSUM dst starts at partition 0 (hardware
    requirement for f32r matmuls): s0 writes channels at psum partitions
    64..64+cout, s255 at 0..cout; the other half of the columns is zero.
    """
    cin = w.shape[1]
    wd = np.transpose(w, (1, 0, 2, 3)).astype(np.float32)
    s0 = np.zeros((3, 128, 128), np.float32)
    s255 = np.zeros((3, 128, 128), np.float32)
    for dx in range(3):
        s0[dx, 0:cin, 64:64 + cout] = wd[:, :, 1, dx]
        s0[dx, 64:64 + cin, 64:64 + cout] = wd[:, :, 2, dx]
        s255[dx, 0:cin, 0:cout] = wd[:, :, 0, dx]
        s255[dx, 64:64 + cin, 0:cout] = wd[:, :, 1, dx]
    return s0, s255


def _pack_l0(w0):
    """w0 [64, 3, 3, 3] -> w0pack [128, 3456] f32 bits.

    L0 image layout: pair-slot j lives at partitions 6*(j%8)..+5, col j//8.
    All matmuls read from partition 0 with leading-zero lhsT rows:
      sec b (b=0..6): [6b+12, 128] = [zeros(6b); A; B]   (one matmul per dx)
      sec 7:          [48, 128]    = [zeros(42); A]      (split pair, part A)
      sec 8:          [6, 128]     = [B]                 (split pair, part B)
    A = [[W0|0],[W1|W0]], B = [[W2|W1],[0|W2]] with W_dy [3, 64] blocks.
    Column layout: sec * 384 + dx * 128.
    """
    wd = np.transpose(w0, (1, 0, 2, 3)).astype(np.float32)  # [3, 64, ky, kx]
    pack = np.zeros((128, 9 * 384), np.float32)

    def ab(dx):
        A = np.zeros((6, 128), np.float32)
        Bm = np.zeros((6, 128), np.float32)
        A[0:3, 0:64] = wd[:, :, 0, dx]
        A[3:6, 0:64] = wd[:, :, 1, dx]
        A[3:6, 64:128] = wd[:, :, 0, dx]
        Bm[0:3, 0:64] = wd[:, :, 2, dx]
        Bm[0:3, 64:128] = wd[:, :, 1, dx]
        Bm[3:6, 64:128] = wd[:, :, 2, dx]
        return A, Bm

    for dx in range(3):
        A, Bm = ab(dx)
        for b in range(7):
            c = b * 384 + dx * 128
            pack[6 * b:6 * b + 6, c:c + 128] = A
            pack[6 * b + 6:6 * b + 12, c:c + 128] = Bm
        c = 7 * 384 + dx * 128
        pack[42:48, c:c + 128] = A
        c = 8 * 384 + dx * 128
        pack[0:6, c:c + 128] = Bm
    return pack


def _pack_all(w0, b0, g0, beta0, mean0, var0, wm, bm, gm, betam, meanm, varm,
              wf, bf, gf, betaf, meanf, varf):
    w0pack = _pack_l0(w0)

    wmid = np.zeros((128, MID * 6 * 128), np.float32)
    for li in range(MID):
        A, Bm = _pair_lhst(wm[li], C)
        for dx in range(3):
            wmid[:, (li * 6 + dx) * 128:(li * 6 + dx + 1) * 128] = A[dx]
            wmid[:, (li * 6 + 3 + dx) * 128:(li * 6 + 4 + dx) * 128] = Bm[dx]

    wfin = np.zeros((128, 12 * 128), np.float32)
    Af, Bf = _pair_lhst(wf, L)
    for dx in range(3):
        wfin[:, dx * 128:(dx + 1) * 128] = Af[dx]
        wfin[:, (3 + dx) * 128:(4 + dx) * 128] = Bf[dx]
    s0f, s255f = _single_lhst(wf, L)
    for dx in range(3):
        wfin[:, (6 + dx) * 128:(7 + dx) * 128] = s0f[dx]
        wfin[:, (9 + dx) * 128:(10 + dx) * 128] = s255f[dx]

    # singles for staggered mid layers L1, L3, L5 = wm[0], wm[2], wm[4]
    wsing = np.zeros((128, 3 * 6 * 128), np.float32)
    for si, li in enumerate([0, 2, 4]):
        s0, s255 = _single_lhst(wm[li], C)
        for dx in range(3):
            wsing[:, (si * 6 + dx) * 128:(si * 6 + dx + 1) * 128] = s0[dx]
            wsing[:, (si * 6 + 3 + dx) * 128:(si * 6 + 4 + dx) * 128] = s255[dx]

    # scale/bias table [128, 16]: col 2l = scale, 2l+1 = bias for layer l
    sbt = np.zeros((128, 16), np.float32)
    s, t = _fold_bn(b0, g0, beta0, mean0, var0)
    sbt[0:C, 0] = s; sbt[64:64 + C, 0] = s
    sbt[0:C, 1] = t; sbt[64:64 + C, 1] = t
    for li in range(MID):
        s, t = _fold_bn(bm[li], gm[li], betam[li], meanm[li], varm[li])
        sbt[0:C, 2 + 2 * li] = s; sbt[64:64 + C, 2 + 2 * li] = s
        sbt[0:C, 3 + 2 * li] = t; sbt[64:64 + C, 3 + 2 * li] = t
    s, t = _fold_bn(bf, gf, betaf, meanf, varf)
    sbt[0:L, 14] = s; sbt[64:64 + L, 14] = s
    sbt[0:L, 15] = t; sbt[64:64 + L, 15] = t

    return w0pack, wmid, wfin, wsing, sbt


# ---------------------------------------------------------------- device build

def build_nc(debug_tap=None, n_images=BPC, max_layer=7):
    """Build the per-core Bass kernel (n_images images). Returns finalized nc.

    debug_tap: None, or int in 0..6 -> after that layer's writes (L0..L6),
    DMA the full main buffer to a debug output (first image only).
    """
    nc = bacc.Bacc("TRN2", target_bir_lowering=False)
    f32r, f32 = dt.float32r, dt.float32

    img = nc.dram_tensor("img", [n_images, CIN, H, W], f32r, kind="ExternalInput")
    w0 = nc.dram_tensor("w0", [128, 3456], f32r, kind="ExternalInput")
    wm = nc.dram_tensor("wm", [128, MID * 6 * 128], f32r, kind="ExternalInput")
    wfn = nc.dram_tensor("wfn", [128, 12 * 128], f32r, kind="ExternalInput")
    wsg = nc.dram_tensor("wsg", [128, 3 * 6 * 128], f32r, kind="ExternalInput")
    zsrc = nc.dram_tensor("zsrc", [128, WPAD], f32r, kind="ExternalInput")
    sbd = nc.dram_tensor("sb", [128, 16], f32, kind="ExternalInput")
    out = nc.dram_tensor("out", [n_images, 128], f32, kind="ExternalOutput")
    if debug_tap is not None:
        dbg = nc.dram_tensor("dbg", [128, NSLOT * WPAD], f32, kind="ExternalOutput")

    with tile.TileContext(nc) as tc:
        with (
            tc.tile_pool(name="big", bufs=1) as big,
            tc.tile_pool(name="ps", bufs=6, space="PSUM") as ps,
        ):
            buf = big.tile([128, NSLOT * WPAD], f32r)
            ibuf = big.tile([128, 17 * WPAD], f32r)
            tw0 = big.tile([128, 3456], f32r)
            twm = big.tile([128, MID * 6 * 128], f32r)
            twf = big.tile([128, 12 * 128], f32r)
            tws = big.tile([128, 3 * 6 * 128], f32r)
            tz = big.tile([128, WPAD], f32r)
            tsb = big.tile([128, 16], f32)
            sums = big.tile([128, 68], f32)
            ostage = big.tile([128, n_images], f32)
            scratch = big.tile([128, 512], f32)

            B3 = buf[:].rearrange("p (s x) -> p s x", x=WPAD)
            I3 = ibuf[:].rearrange("p (s x) -> p s x", x=WPAD)

            nc.sync.dma_start(tz[:], zsrc[:])
            tzb = tz[:].rearrange("p (o x) -> p o x", o=1)
            nc.sync.dma_start(B3[:, :, :], tzb.broadcast_to([128, NSLOT, WPAD]))
            nc.sync.dma_start(I3[:, :, :], tzb.broadcast_to([128, 17, WPAD]))
            nc.sync.dma_start(tw0[:], w0[:])
            nc.sync.dma_start(twm[:], wm[:])
            nc.sync.dma_start(twf[:], wfn[:])
            nc.sync.dma_start(tws[:], wsg[:])
            nc.sync.dma_start(tsb[:], sbd[:])

            def scale_of(l):
                return tsb[:, 2 * l:2 * l + 1]

            def bias_of(l):
                return tsb[:, 2 * l + 1:2 * l + 2]

            RELU = mybir.ActivationFunctionType.Relu

            def mid_lhst(li, ab, dx):  # li 0..5 for L1..L6
                c = (li * 6 + ab * 3 + dx) * 128
                return twm[:, c:c + 128]

            def fin_lhst(ab, dx):
                c = (ab * 3 + dx) * 128
                return twf[:, c:c + 128]

            def sing_lhst(layer, which, dx):  # layer in {1,3,5}, which 0=row0 1=row255
                si = {1: 0, 3: 1, 5: 2}[layer]
                c = (si * 6 + which * 3 + dx) * 128
                return tws[:, c:c + 128]

            def fin_sing_lhst(which, dx):
                c = (6 + which * 3 + dx) * 128
                return twf[:, c:c + 128]

            # ---------------- layer emitters ----------------

            def emit_l0(im):
                # image load: 16 DMAs into 8-subblock layout
                for b in range(8):
                    j0 = b if b > 0 else 8
                    r0 = 2 * j0 - 1
                    nb = (128 - j0) // 8 + 1
                    nc.sync.dma_start(
                        I3[6 * b:6 * b + 3, j0 // 8:j0 // 8 + nb, 1:257],
                        img[im, :, r0:256:16, :],
                    )
                    r0e = 2 * b
                    nbe = (127 - b) // 8 + 1
                    nc.sync.dma_start(
                        I3[6 * b + 3:6 * b + 6, 0:nbe, 1:257],
                        img[im, :, r0e:256:16, :],
                    )
                # 64 groups of 2 out-pairs
                for g in range(64):
                    pt = ps.tile([128, 512], f32, tag="acc")
                    pt3 = pt[:].rearrange("p (s x) -> p s x", x=256)
                    for h in range(2):
                        k = 2 * g + h
                        b = k % 8
                        col = k // 8
                        po = pt[:, h * 256:(h + 1) * 256]
                        if b < 7:
                            kk = 6 * b + 12
                            for dx in range(3):
                                c = b * 384 + dx * 128
                                nc.tensor.matmul(
                                    po, tw0[0:kk, c:c + 128],
                                    I3[0:kk, col, dx:dx + 256],
                                    start=(dx == 0), stop=(dx == 2))
                        else:
                            for dx in range(3):
                                ca = 7 * 384 + dx * 128
                                cb = 8 * 384 + dx * 128
                                nc.tensor.matmul(
                                    po, tw0[0:48, ca:ca + 128],
                                    I3[0:48, col, dx:dx + 256],
                                    start=(dx == 0), stop=False)
                                nc.tensor.matmul(
                                    po, tw0[0:6, cb:cb + 128],
                                    I3[0:6, col + 1, dx:dx + 256],
                                    start=False, stop=(dx == 2))
                    # out pairs 2g, 2g+1 -> B-layout offset 3: phys 2g+3, 2g+4
                    nc.scalar.activation(
                        B3[:, 2 * g + 3:2 * g + 5, 1:257], pt3,
                        RELU, bias=bias_of(0), scale=scale_of(0))

            def emit_clean(lnum, li, o):
                # input A-layout at phys o, output B-layout at phys o
                for g in range(64):
                    pt = ps.tile([128, 512], f32, tag="acc")
                    pt3 = pt[:].rearrange("p (s x) -> p s x", x=256)
                    for dx in range(3):
                        nc.tensor.matmul(
                            pt[:], mid_lhst(li, 0, dx),
                            B3[:, o + 2 * g:o + 2 * g + 2, dx:dx + 256],
                            start=(dx == 0), stop=False)
                    for dx in range(3):
                        nc.tensor.matmul(
                            pt[:], mid_lhst(li, 1, dx),
                            B3[:, o + 2 * g + 1:o + 2 * g + 3, dx:dx + 256],
                            start=False, stop=(dx == 2))
                    nc.scalar.activation(
                        B3[:, o + 2 * g:o + 2 * g + 2, 1:257], pt3,
                        RELU, bias=bias_of(lnum), scale=scale_of(lnum))

            def emit_stag(lnum, li, o_in, o_out):
                # input B-layout at phys o_in, output A-layout at phys o_out
                # pairs k=0..126; groups g=0..62 (2 pairs), leftover k=126
                for g in range(63):
                    pt = ps.tile([128, 512], f32, tag="acc")
                    pt3 = pt[:].rearrange("p (s x) -> p s x", x=256)
                    for dx in range(3):
                        nc.tensor.matmul(
                            pt[:], mid_lhst(li, 0, dx),
                            B3[:, o_in + 2 * g:o_in + 2 * g + 2, dx:dx + 256],
                            start=(dx == 0), stop=False)
                    for dx in range(3):
                        nc.tensor.matmul(
                            pt[:], mid_lhst(li, 1, dx),
                            B3[:, o_in + 2 * g + 1:o_in + 2 * g + 3, dx:dx + 256],
                            start=False, stop=(dx == 2))
                    nc.scalar.activation(
                        B3[:, o_out + 2 * g + 1:o_out + 2 * g + 3, 1:257], pt3,
                        RELU, bias=bias_of(lnum), scale=scale_of(lnum))
                # leftover pair k=126
                pt = ps.tile([128, 512], f32, tag="acc")
                for dx in range(3):
                    nc.tensor.matmul(
                        pt[:, 0:256], mid_lhst(li, 0, dx),
                        B3[:, o_in + 126, dx:dx + 256],
                        start=(dx == 0), stop=False)
                for dx in range(3):
                    nc.tensor.matmul(
                        pt[:, 0:256], mid_lhst(li, 1, dx),
                        B3[:, o_in + 127, dx:dx + 256],
                        start=False, stop=(dx == 2))
                nc.scalar.activation(
                    B3[:, o_out + 127, 1:257], pt[:, 0:256],
                    RELU, bias=bias_of(lnum), scale=scale_of(lnum))
                # single row 0 -> A-slot 0 (phys o_out) partitions 64..127
                pt = ps.tile([128, 512], f32, tag="acc")
                for dx in range(3):
                    nc.tensor.matmul(
                        pt[:, 0:256], sing_lhst(lnum, 0, dx),
                        B3[:, o_in + 0, dx:dx + 256],
                        start=(dx == 0), stop=(dx == 2))
                nc.scalar.activation(
                    B3[64:128, o_out + 0, 1:257], pt[64:128, 0:256],
                    RELU, bias=bias_of(lnum)[64:128], scale=scale_of(lnum)[64:128])
                # single row 255 -> A-slot 128 (phys o_out+128) partitions 0..63
                pt = ps.tile([128, 512], f32, tag="acc")
                for dx in range(3):
                    nc.tensor.matmul(
                        pt[:, 0:256], sing_lhst(lnum, 1, dx),
                        B3[:, o_in + 127, dx:dx + 256],
                        start=(dx == 0), stop=(dx == 2))
                nc.scalar.activation(
                    B3[0:64, o_out + 128, 1:257], pt[0:64, 0:256],
                    RELU, bias=bias_of(lnum)[0:64], scale=scale_of(lnum)[0:64])
                # re-zero pad: input B-slot 127 (phys o_in+127) partitions 64..127
                # becomes "row 256" pad of the A-layout the next layer reads.
                nc.sync.dma_start(B3[64:128, o_in + 127, 0:WPAD], tz[64:128, :])

            def emit_final(im, o_in):
                lnum = 7
                ncol = 0
                for g in range(63):
                    pt = ps.tile([128, 512], f32, tag="acc")
                    pt3 = pt[:].rearrange("p (s x) -> p s x", x=256)
                    for dx in range(3):
                        nc.tensor.matmul(
                            pt[:], fin_lhst(0, dx),
                            B3[:, o_in + 2 * g:o_in + 2 * g + 2, dx:dx + 256],
                            start=(dx == 0), stop=False)
                    for dx in range(3):
                        nc.tensor.matmul(
                            pt[:], fin_lhst(1, dx),
                            B3[:, o_in + 2 * g + 1:o_in + 2 * g + 3, dx:dx + 256],
                            start=False, stop=(dx == 2))
                    sc3 = scratch[:].rearrange("p (s x) -> p s x", x=256)
                    nc.scalar.activation(
                        sc3, pt3, RELU,
                        bias=bias_of(lnum), scale=scale_of(lnum),
                        accum_out=sums[:, ncol:ncol + 1])
                    ncol += 1
                # leftover pair k=126
                pt = ps.tile([128, 512], f32, tag="acc")
                for dx in range(3):
                    nc.tensor.matmul(
                        pt[:, 0:256], fin_lhst(0, dx),
                        B3[:, o_in + 126, dx:dx + 256],
                        start=(dx == 0), stop=False)
                for dx in range(3):
                    nc.tensor.matmul(
                        pt[:, 0:256], fin_lhst(1, dx),
                        B3[:, o_in + 127, dx:dx + 256],
                        start=False, stop=(dx == 2))
                nc.scalar.activation(
                    scratch[:, 0:256], pt[:, 0:256], RELU,
                    bias=bias_of(lnum), scale=scale_of(lnum),
                    accum_out=sums[:, ncol:ncol + 1])
                ncol += 1
                # single row 0 (partitions 64..127)
                pt = ps.tile([128, 512], f32, tag="acc")
                for dx in range(3):
                    nc.tensor.matmul(
                        pt[:, 0:256], fin_sing_lhst(0, dx),
                        B3[:, o_in + 0, dx:dx + 256],
                        start=(dx == 0), stop=(dx == 2))
                nc.scalar.activation(
                    scratch[64:128, 0:256], pt[64:128, 0:256], RELU,
                    bias=bias_of(lnum)[64:128], scale=scale_of(lnum)[64:128],
                    accum_out=sums[64:128, ncol:ncol + 1])
                ncol += 1
                # single row 255 (partitions 0..63)
                pt = ps.tile([128, 512], f32, tag="acc")
                for dx in range(3):
                    nc.tensor.matmul(
                        pt[:, 0:256], fin_sing_lhst(1, dx),
                        B3[:, o_in + 127, dx:dx + 256],
                        start=(dx == 0), stop=(dx == 2))
                nc.scalar.activation(
                    scratch[0:64, 0:256], pt[0:64, 0:256], RELU,
                    bias=bias_of(lnum)[0:64], scale=scale_of(lnum)[0:64],
                    accum_out=sums[0:64, ncol:ncol + 1])
                ncol += 1
                # reduce all accum columns -> per-channel sums for this image
                nc.vector.tensor_reduce(
                    ostage[:, im:im + 1], sums[:, 0:ncol],
                    axis=mybir.AxisListType.X, op=mybir.AluOpType.add)
                nc.sync.dma_start(out[im, :], ostage[:, im:im + 1])

            # ---------------- main program ----------------
            emitters = [
                lambda im: emit_l0(im),
                lambda im: emit_stag(1, 0, 3, 2),
                lambda im: emit_clean(2, 1, 2),
                lambda im: emit_stag(3, 2, 2, 1),
                lambda im: emit_clean(4, 3, 1),
                lambda im: emit_stag(5, 4, 1, 0),
                lambda im: emit_clean(6, 5, 0),
                lambda im: emit_final(im, 0),
            ]
            for im in range(n_images):
                # cross-image pad re-zeroing (stale from previous image)
                nc.sync.dma_start(B3[0:64, 1, 0:WPAD], tz[0:64, :])
                nc.sync.dma_start(B3[0:64, 2, 0:WPAD], tz[0:64, :])
                nc.vector.memset(sums[:], 0.0)
                for lyr in range(0, max_layer + 1):
                    emitters[lyr](im)
                    if debug_tap == lyr and im == 0:
                        nc.sync.dma_start(dbg[:], buf[:].bitcast(f32))
                if max_layer < 7:
                    # keep "out" written so the output exists
                    nc.vector.memset(ostage[:, im:im + 1], 0.0)
                    nc.sync.dma_start(out[im, :], ostage[:, im:im + 1])

    nc.finalize()
    return nc


# ---------------------------------------------------------------- entry point

_CACHE = {}


def _get_runner():
    if "fn" in _CACHE:
        return _CACHE["fn"], _CACHE["in_names"]
    nc = build_nc()
    import jax
    from jax.sharding import Mesh, PartitionSpec
    from jax.experimental.shard_map import shard_map
    from concourse import mybir as _mb
    from concourse.bass2jax import (
        _bass_exec_p, partition_id_tensor, install_neuronx_cc_hook)

    install_neuronx_cc_hook()
    # surface swallowed compile-hook exceptions
    import libneuronxla, traceback
    _real_ncc = libneuronxla.neuronx_cc
    def _ncc_wrapped(*a, **kw):
        try:
            return _real_ncc(*a, **kw)
        except BaseException:
            traceback.print_exc()
            with open("/tmp/ncc_hook_error.log", "w") as f:
                traceback.print_exc(file=f)
            raise
    libneuronxla.neuronx_cc = _ncc_wrapped
    partition_name = nc.partition_id_tensor.name if nc.partition_id_tensor else None

    in_names, out_names, out_avals, zero_outs = [], [], [], []
    for alloc in nc.m.functions[0].allocations:
        if not isinstance(alloc, _mb.MemoryLocationSet):
            continue
        name = alloc.memorylocations[0].name
        if alloc.kind == "ExternalInput":
            if name != partition_name:
                in_names.append(name)
        elif alloc.kind == "ExternalOutput":
            shape = tuple(alloc.tensor_shape)
            dtype = _mb.dt.np(alloc.dtype)
            out_avals.append(jax.core.ShapedArray(shape, dtype))
            out_names.append(name)
            zero_outs.append(np.zeros(shape, dtype))

    n_params = len(in_names)
    n_outs = len(out_avals)
    all_in_names = list(in_names) + list(out_names)
    if partition_name is not None:
        all_in_names.append(partition_name)

    def _body(*args):
        operands = list(args)
        if partition_name is not None:
            operands.append(partition_id_tensor())
        outs = _bass_exec_p.bind(
            *operands,
            out_avals=tuple(out_avals),
            in_names=tuple(all_in_names),
            out_names=tuple(out_names),
            lowering_input_output_aliases=(),
            sim_require_finite=True,
            sim_require_nnan=True,
            nc=nc,
        )
        return tuple(outs)

    devices = jax.devices()[:NCORES]
    mesh = Mesh(np.asarray(devices), ("core",))
    in_specs = (PartitionSpec("core"),) * (n_params + n_outs)
    out_specs = (PartitionSpec("core"),) * n_outs
    jitted = jax.jit(
        shard_map(_body, mesh=mesh, in_specs=in_specs, out_specs=out_specs,
                  check_rep=False),
        keep_unused=True,
    )

    def fn(concat_inputs):
        concat_zeros = [
            np.zeros((NCORES * z.shape[0], *z.shape[1:]), z.dtype)
            for z in zero_outs
        ]
        out_arrs = jitted(*concat_inputs, *concat_zeros)
        return [np.asarray(o) for o in out_arrs]

    _CACHE["fn"] = fn
    _CACHE["in_names"] = in_names
    return fn, in_names


def kernel(image_with_wm, w0, b0, g0, beta0, mean0, var0,
           wm, bm, gm, betam, meanm, varm,
           wf, bf, gf, betaf, meanf, varf):
    image_with_wm = np.ascontiguousarray(np.asarray(image_with_wm, np.float32))
    packs = _pack_all(
        np.asarray(w0, np.float32), np.asarray(b0, np.float32),
        np.asarray(g0, np.float32), np.asarray(beta0, np.float32),
        np.asarray(mean0, np.float32), np.asarray(var0, np.float32),
        np.asarray(wm, np.float32), np.asarray(bm, np.float32),
        np.asarray(gm, np.float32), np.asarray(betam, np.float32),
        np.asarray(meanm, np.float32), np.asarray(varm, np.float32),
        np.asarray(wf, np.float32), np.asarray(bf, np.float32),
        np.asarray(gf, np.float32), np.asarray(betaf, np.float32),
        np.asarray(meanf, np.float32), np.asarray(varf, np.float32))
    w0pack, wmid, wfin, wsing, sbt = packs

    fn, in_names = _get_runner()
    zsrc = np.zeros((128, WPAD), np.float32)
    per_core = {
        "img": [image_with_wm[c * BPC:(c + 1) * BPC] for c in range(NCORES)],
        "zsrc": [zsrc] * NCORES,
        "w0": [w0pack] * NCORES,
        "wm": [wmid] * NCORES,
        "wfn": [wfin] * NCORES,
        "wsg": [wsing] * NCORES,
        "sb": [sbt] * NCORES,
    }
    concat_inputs = [
        np.ascontiguousarray(np.concatenate(per_core[name], axis=0))
        for name in in_names
    ]
    outs = fn(concat_inputs)
    acc = outs[0].reshape(NCORES * BPC, 128)  # out is first (only) output
    msg = (acc[:, 0:L] + acc[:, 64:64 + L]) * np.float32(1.0 / (H * W))
    return msg.astype(np.float32)
